# revision 16
# baseline (speedup 1.0000x reference)
"""BiLevelRoutingAttention (spiking) Trainium2 kernel, v2.

Sharding: one (t, b) pair per core (T=4 x B=2 = 8 cores). Routing region mean
is an AllReduce among the 4 cores sharing each b.

v2 vs v1:
  - fp8e4 DoubleRow matmuls for qkv / projection (0.5 cyc/col; the 2-way
    channel-half contraction folds into one instruction). Grams also fp8 DR.
  - spike thresholds normalized into the weights (x@w+b >= 2  <=>
    x@(128*w/(2-b)) >= 128), so thresholding is a constant-scalar compare
    split across DVE (is_ge) and ScalarE (saturated sigmoid), one batched
    [128,1024] drain per PSUM pair.
  - gram combine stays on-chip: SBUF->SBUF DMA transposes to window-major and
    back; block-diag bmask folded into the gram PSUM drain.
  - attention divide: rank-1 eps matmul folds +1e-6 into den, DVE
    reciprocal_approx_fast from PSUM, ScalarE copy + GpSimd multiply.
  - projection result DMAd straight from PSUM as f32; host applies
    /64 (fp8 weight scale) and + b_proj.
"""

import numpy as np
import ml_dtypes

T, B, Lt, Lh, Lw, C = 4, 2, 8, 32, 32, 256
WT, WH, WW = 2, 4, 4
LT, LH, LW = Lt // WT, Lh // WH, Lw // WW  # 4, 8, 8
W = WT * WH * WW        # 32 windows
S = LT * LH * LW        # 256 tokens per window
NTOK = W * S            # 8192
H, D = 8, 32
TOPK = 4
NCORES = 8
GROUPS = [[0, 1, 2, 3], [4, 5, 6, 7]]
BF16 = ml_dtypes.bfloat16
WSCALE = 128.0          # spike thresholds scaled to this constant
PSCALE = 64.0           # w_proj scaled by this (undone on host)
SIGK = 1e6              # sigmoid sharpness for ScalarE spike compare
CCH = 344               # combine chunk cols (8256 = 24*344)

_CACHE = {}


def build_kernel():
    from concourse import bacc
    import concourse.mybir as mybir
    import concourse.tile as tile
    from concourse.tile_rust import add_dep_helper
    from concourse.masks import make_identity

    bf = mybir.dt.bfloat16
    f32 = mybir.dt.float32
    f8 = mybir.dt.float8e4
    DR = mybir.MatmulPerfMode.DoubleRow

    nc = bacc.Bacc("TRN2", target_bir_lowering=False, debug=False,
                   num_devices=NCORES)

    xT = nc.dram_tensor("xT", [2, 128, NTOK], bf, kind="ExternalInput")
    x8d = nc.dram_tensor("x8", [2, 128, NTOK], f8, kind="ExternalInput")
    wq = nc.dram_tensor("wq", [128, 2, 2, 128], f8, kind="ExternalInput")
    wkv = nc.dram_tensor("wkv", [128, 2, 512], f8, kind="ExternalInput")
    wproj = nc.dram_tensor("wproj", [128, 2, 2, 128], f8, kind="ExternalInput")
    bmask = nc.dram_tensor("bmask", [128, 2, 129], bf, kind="ExternalInput")
    outT = nc.dram_tensor("outT", [2, 128, NTOK], bf, kind="ExternalOutput")
    sel_dbg = nc.dram_tensor("sel_dbg", [32, 32], f32, kind="ExternalOutput")

    cc_in = nc.dram_tensor("cc_in", [128, 64], f32)
    cc_out = nc.dram_tensor("cc_out", [128, 64], f32)

    with tile.TileContext(nc) as tc:
        with (
            tc.tile_pool(name="big", bufs=1) as big_pool,
            tc.tile_pool(name="persist", bufs=1) as pp,
            tc.tile_pool(name="kvs", bufs=4) as kv_pool,
            tc.tile_pool(name="small", bufs=2) as sm_pool,
            tc.tile_pool(name="outp", bufs=3) as out_pool,
            tc.tile_pool(name="mm512", bufs=2, space="PSUM") as mm512,
            tc.tile_pool(name="big2", bufs=3, space="PSUM") as bp2,
            tc.tile_pool(name="dram", bufs=1, space="DRAM") as dram_pool,
        ):
            gram_dram = dram_pool.tile([32, 128, 258], bf)
            kvr_dram = dram_pool.tile([32, 128, 258], bf)
            # ---- big persistent SBUF tiles ----
            xsb = big_pool.tile([128, 2, NTOK], bf, tag="xsb")
            x8 = big_pool.tile([128, 2, NTOK], f8, tag="x8")
            qsb = big_pool.tile([128, 2, NTOK], bf, tag="qsb")
            attn_nb = big_pool.tile([128, 2, NTOK], f8, tag="attnb")
            gram_all = big_pool.tile([128, 32, 258], bf, tag="gram")
            wmaj = big_pool.tile([128, 8256], bf, tag="wmaj")
            kvr_wmaj = big_pool.tile([128, 8256], bf, tag="kvrw")
            kvread = big_pool.tile([128, 32, 258], bf, tag="kvread")
            dexp = big_pool.tile([128, 2, 32, 128], bf, tag="dexp")

            # ---- input DMAs (sliced so compute can start early) ----
            for p in range(4):
                sl = slice(p * 2048, (p + 1) * 2048)
                for c in range(2):
                    nc.sync.dma_start(x8[:, c, sl], x8d[c, :, sl])
                for c in range(2):
                    nc.sync.dma_start(xsb[:, c, sl], xT[c, :, sl])

            # ---- weights / constants ----
            wq_sb = pp.tile([128, 2, 2, 128], f8)
            nc.sync.dma_start(wq_sb[:], wq[:])
            wkv_sb = pp.tile([128, 2, 512], f8)
            nc.sync.dma_start(wkv_sb[:], wkv[:])
            wproj_sb = pp.tile([128, 2, 2, 128], f8)
            nc.sync.dma_start(wproj_sb[:], wproj[:])
            bmask_sb = pp.tile([128, 2, 129], bf)
            nc.sync.dma_start(bmask_sb[:], bmask[:])
            id32 = pp.tile([32, 32], f32)
            make_identity(nc, id32[:])
            sigb = pp.tile([128, 1], f32)
            nc.vector.memset(sigb[:], -WSCALE * SIGK)
            epsrow = pp.tile([1, 128], bf)
            nc.vector.memset(epsrow[:], 1e-6)
            ones256 = pp.tile([1, 256], bf)
            nc.vector.memset(ones256[:], 1.0)

            # ---- region partial sums (per x slice) -> collective ----
            region = sm_pool.tile([128, 2, 32], f32, tag="region", bufs=1)
            for p in range(4):
                sl = slice(p * 2048, (p + 1) * 2048)
                for c in range(2):
                    nc.vector.reduce_sum(
                        region[:, c, p * 8:(p + 1) * 8],
                        xsb[:, c, sl].rearrange("p (w s) -> p w s", s=S),
                        axis=mybir.AxisListType.X,
                    )
            st = nc.sync.dma_start(cc_in[:], region[:].rearrange("p a w -> p (a w)"))
            cc = nc.gpsimd.collective_compute(
                "AllReduce", mybir.AluOpType.add, replica_groups=GROUPS,
                ins=[cc_in[:]], outs=[cc_out[:]],
            )
            add_dep_helper(cc.ins, st.ins, reason="region stored before collective")
            xs_sb = sm_pool.tile([128, 2, 32], f32, tag="xsum", bufs=1)
            ld = nc.sync.dma_start(xs_sb[:], cc_out[:].rearrange("p (a w) -> p a w", w=32))
            add_dep_helper(ld.ins, cc.ins, reason="collective before readback")

            bmflat = bmask_sb[:].rearrange("p c e -> p (c e)")

            # ---- qkv + spikes + per-window grams (16 blocks of 512 toks) ----
            for blk in range(16):
                tsl = slice(blk * 512, (blk + 1) * 512)
                # q: both halves into one 2-bank PSUM tile, one DVE drain
                qp = bp2.tile([128, 1024], f32, tag="big2")
                for qc in range(2):
                    nc.tensor.matmul(qp[:, 512 * qc:512 * (qc + 1)],
                                     wq_sb[:, :, qc, :], x8[:, :, tsl],
                                     start=True, stop=True, perf_mode=DR)
                nc.vector.tensor_scalar(
                    qsb[:, :, tsl],
                    qp[:].rearrange("p (a t) -> p a t", a=2),
                    WSCALE, None, op0=mybir.AluOpType.is_ge)
                # kv: per window, 2 token-chunk matmuls into one tile,
                # one ScalarE sigmoid drain into the fp8 kvt tile
                kvts = {}
                for wi in range(2):
                    kvt = kv_pool.tile([128, 2, 528], f8, tag="kvt")
                    kvts[wi] = kvt
                    nc.gpsimd.memset(kvt[:, :, 512:513], 1.0)
                    kvp = bp2.tile([128, 1024], f32, tag="big2")
                    for ci in range(2):
                        tci = blk * 4 + wi * 2 + ci
                        ksl = slice(tci * 128, (tci + 1) * 128)
                        nc.tensor.matmul(kvp[:, 512 * ci:512 * (ci + 1)],
                                         x8[:, :, ksl], wkv_sb[:],
                                         start=True, stop=True, perf_mode=DR)
                    nc.scalar.activation(
                        kvt[:, :, 0:512],
                        kvp[:].rearrange("p (a t) -> p a t", a=2),
                        mybir.ActivationFunctionType.Sigmoid,
                        bias=sigb[:], scale=SIGK)
                # grams for the two windows into one tile, one masked drain
                gp = bp2.tile([128, 1024], f32, tag="big2")
                for wi in range(2):
                    kvt = kvts[wi]
                    for c in range(2):
                        off = 512 * wi + 129 * c
                        nc.tensor.matmul(gp[:, off:off + 128],
                                         kvt[:, :, 128 * c:128 * (c + 1)],
                                         kvt[:, :, 256 + 128 * c:384 + 128 * c],
                                         start=True, stop=True, perf_mode=DR)
                        nc.tensor.matmul(gp[:, off + 128:off + 129],
                                         kvt[:, :, 128 * c:128 * (c + 1)],
                                         kvt[:, :, 512:513],
                                         start=True, stop=True, perf_mode=DR)
                nc.vector.tensor_tensor(
                    gram_all[:, 2 * blk:2 * blk + 2, :],
                    gp[:].rearrange("p (w x) -> p w x", w=2)[:, :, 0:258],
                    bmflat[:, None, :].to_broadcast([128, 2, 258]),
                    op=mybir.AluOpType.mult)
                nc.sync.dma_start(
                    gram_dram[2 * blk:2 * blk + 2, :, :].rearrange("w p e -> p w e"),
                    gram_all[:, 2 * blk:2 * blk + 2, :])

            # ---- scores -> top-4 selection matrix sel^T (replicated x4) ----
            scpt = mm512.tile([128, 512], f32, tag="mm512")
            scp = scpt[0:32, :]
            for c in range(2):
                nc.tensor.matmul(scp[:, 0:32], xs_sb[:, c, :], xs_sb[:, c, :],
                                 start=(c == 0), stop=(c == 1))
            shifted = sm_pool.tile([32, 32], f32, tag="shifted")
            nc.vector.tensor_scalar(shifted[:], scp[:, 0:32], 1e6, None,
                                    op0=mybir.AluOpType.add)
            mx8 = sm_pool.tile([32, 8], f32, tag="mx8")
            nc.vector.max(mx8[:], shifted[:])
            nc.vector.memset(mx8[:, TOPK:], 0.0)
            zapped = sm_pool.tile([32, 32], f32, tag="zapped")
            nc.vector.match_replace(out=zapped[:], in_to_replace=mx8[:],
                                    in_values=shifted[:], imm_value=0.0)
            selb = sm_pool.tile([32, 32], f32, tag="selb")
            nc.vector.tensor_tensor(selb[:], shifted[:], zapped[:],
                                    op=mybir.AluOpType.is_gt)
            nc.sync.dma_start(sel_dbg[:], selb[:])
            selT_pt = mm512.tile([128, 512], f32, tag="mm512")
            nc.tensor.transpose(selT_pt[0:32, 0:32], selb[:], id32[:])
            selT = sm_pool.tile([32, 32], bf, tag="selT")
            nc.vector.tensor_copy(selT[:], selT_pt[0:32, 0:32])
            selT4 = pp.tile([128, 32], bf)
            for j in range(4):
                nc.sync.dma_start(selT4[32 * j:32 * (j + 1), :], selT[:])

            # ---- window-major gram read (via DRAM round trip) ----
            for a in range(4):
                psl = slice(32 * a, 32 * (a + 1))
                nc.sync.dma_start(
                    wmaj[psl, :].rearrange("j (pl e) -> j pl e", e=258),
                    gram_dram[:, psl, :])

            # ---- combine: kvr rows = selT.T @ gram rows (4-block packed) ----
            for grp in range(24):
                csl = slice(grp * CCH, (grp + 1) * CCH)
                cpt = mm512.tile([128, 512], f32, tag="mm512")
                cp = cpt[:, 0:CCH]
                for a in range(4):
                    psl = slice(32 * a, 32 * (a + 1))
                    nc.tensor.matmul(cp[psl, :], selT4[psl, :], wmaj[psl, csl],
                                     start=True, stop=True,
                                     tile_position=(32 * a, 32 * a))
                if grp % 2 == 0:
                    nc.vector.tensor_copy(kvr_wmaj[:, csl], cp[:])
                else:
                    nc.scalar.activation(kvr_wmaj[:, csl], cp[:],
                                         mybir.ActivationFunctionType.Copy)

            # ---- transpose back via DRAM: kvread[kd, w, (c e)] ----
            for a in range(4):
                psl = slice(32 * a, 32 * (a + 1))
                nc.sync.dma_start(
                    kvr_dram[:, psl, :],
                    kvr_wmaj[psl, :].rearrange("w (pl e) -> w pl e", e=258))
            for a in range(4):
                psl = slice(32 * a, 32 * (a + 1))
                nc.sync.dma_start(
                    kvread[psl, :, :],
                    kvr_dram[:, psl, :].rearrange("w p e -> p w e"))

            # ---- dexp: ksum column broadcast, block-diag masked ----
            for c in range(2):
                nc.vector.tensor_tensor(
                    dexp[:, c, :, :],
                    kvread[:, :, 129 * c + 128:129 * c + 129].to_broadcast([128, 32, 128]),
                    bmask_sb[:, 0:1, 0:128].to_broadcast([128, 32, 128]),
                    op=mybir.AluOpType.mult,
                )

            # ---- attention (out, den) + divide; projection; out DMA ----
            for blk in range(16):
                tsl = slice(blk * 512, (blk + 1) * 512)
                adp_o = bp2.tile([128, 1024], f32, tag="big2")
                adp_d = bp2.tile([128, 1024], f32, tag="big2")
                for wi in range(2):
                    w = blk * 2 + wi
                    wsl = slice(w * 256, (w + 1) * 256)
                    for c in range(2):
                        qtr = slice(512 * wi + 256 * c, 512 * wi + 256 * (c + 1))
                        nc.tensor.matmul(adp_o[:, qtr],
                                         kvread[:, w, 129 * c:129 * c + 128],
                                         qsb[:, c, wsl], start=True, stop=True)
                        nc.tensor.matmul(adp_d[:, qtr], dexp[:, c, w, :],
                                         qsb[:, c, wsl], start=True, stop=False)
                        nc.tensor.matmul(adp_d[:, qtr], epsrow[:],
                                         ones256[:], start=False, stop=True)
                rsb = out_pool.tile([128, 1024], f32, tag="rsb")
                nc.vector.reciprocal_approx_fast(out=rsb[:], in_=adp_d[:])
                attc = out_pool.tile([128, 1024], f8, tag="attc")
                nc.scalar.activation(attc[:], adp_o[:],
                                     mybir.ActivationFunctionType.Copy)
                nc.gpsimd.tensor_tensor(
                    attn_nb[:, :, tsl].rearrange("p c (w s) -> p w c s", w=2),
                    attc[:].rearrange("p (w c s) -> p w c s", w=2, c=2),
                    rsb[:].rearrange("p (w c s) -> p w c s", w=2, c=2),
                    op=mybir.AluOpType.mult)
                for pc in range(2):
                    pjp = mm512.tile([128, 512], f32, tag="mm512")
                    nc.tensor.matmul(pjp[:], wproj_sb[:, :, pc, :],
                                     attn_nb[:, :, tsl],
                                     start=True, stop=True, perf_mode=DR)
                    osb = out_pool.tile([128, 512], bf, tag="osb")
                    if pc == 0:
                        nc.scalar.activation(osb[:], pjp[:],
                                             mybir.ActivationFunctionType.Copy)
                    else:
                        nc.vector.tensor_copy(osb[:], pjp[:])
                    nc.sync.dma_start(outT[pc, :, tsl], osb[:])

    nc.compile()
    return nc


def prep_inputs(x, w_qkv, b_qkv, w_proj, b_proj):
    from concourse import mybir
    f8np = mybir.dt.np(mybir.dt.float8e4)
    th = 2.0 - b_qkv                       # all positive (~2)
    wn = w_qkv * (WSCALE / th)             # spike threshold becomes WSCALE
    wq_a = wn[:, 0:256].reshape(2, 128, 2, 128).transpose(1, 0, 2, 3)
    wkv_a = wn[:, 256:768].reshape(2, 128, 512).transpose(1, 0, 2)
    wproj_a = (w_proj * PSCALE).reshape(2, 128, 2, 128).transpose(1, 0, 2, 3)
    i = np.arange(128)[:, None]
    j = np.arange(129)[None, :]
    bm = ((i // 32) == (j // 32)) | (j == 128)
    bmask_a = np.broadcast_to(bm[:, None, :], (128, 2, 129))
    return {
        "wq": np.ascontiguousarray(wq_a).astype(f8np),
        "wkv": np.ascontiguousarray(wkv_a).astype(f8np),
        "wproj": np.ascontiguousarray(wproj_a).astype(f8np),
        "bmask": np.ascontiguousarray(bmask_a).astype(BF16),
    }, f8np


def window_partition(x):
    """[T,B,Lt,Lh,Lw,C] -> [T,B,NTOK,C] with tokens in (w, s) order."""
    Tb, Bb = x.shape[0], x.shape[1]
    xw = x.reshape(Tb, Bb, WT, LT, WH, LH, WW, LW, C)
    xw = xw.transpose(0, 1, 2, 4, 6, 3, 5, 7, 8)
    return np.ascontiguousarray(xw).reshape(Tb, Bb, NTOK, C)


def window_reverse(o):
    """[NTOK, C] -> [Lt, Lh, Lw, C]."""
    o = o.reshape(WT, WH, WW, LT, LH, LW, C)
    o = o.transpose(0, 3, 1, 4, 2, 5, 6)
    return np.ascontiguousarray(o).reshape(Lt, Lh, Lw, C)


def make_in_maps(inputs):
    x = np.asarray(inputs["x"], dtype=np.float32)
    shared, f8np = prep_inputs(
        x, np.asarray(inputs["w_qkv"], dtype=np.float32),
        np.asarray(inputs["b_qkv"], dtype=np.float32),
        np.asarray(inputs["w_proj"], dtype=np.float32),
        np.asarray(inputs["b_proj"], dtype=np.float32))
    xw = window_partition(x)
    in_maps = []
    for core in range(NCORES):
        b, t = core // 4, core % 4
        xt = np.ascontiguousarray(xw[t, b].T)              # [C, NTOK]
        in_maps.append({
            **shared,
            "xT": xt.reshape(2, 128, NTOK).astype(BF16),
            "x8": xt.reshape(2, 128, NTOK).astype(f8np),
        })
    return in_maps


def run_kernel_spmd(nc, in_maps, **kwargs):
    from concourse.bass_utils import run_bass_kernel_spmd
    return run_bass_kernel_spmd(nc, in_maps, core_ids=list(range(NCORES)), **kwargs)


def collect_out(res, b_proj):
    out = np.empty((T, B, Lt, Lh, Lw, C), dtype=np.float32)
    bp = np.asarray(b_proj, dtype=np.float32)
    for core in range(NCORES):
        b, t = core // 4, core % 4
        oT = res.results[core]["outT"].astype(np.float32).reshape(256, NTOK)
        o = np.ascontiguousarray(oT.T) * (1.0 / PSCALE) + bp
        out[t, b] = window_reverse(o)
    return out


def kernel(x, w_qkv, b_qkv, w_proj, b_proj):
    if "nc" not in _CACHE:
        _CACHE["nc"] = build_kernel()
    nc = _CACHE["nc"]
    in_maps = make_in_maps(dict(x=x, w_qkv=w_qkv, b_qkv=b_qkv,
                                w_proj=w_proj, b_proj=b_proj))
    res = run_kernel_spmd(nc, in_maps)
    return collect_out(res, b_proj)


# revision 17
# speedup vs baseline: 1.2404x; 1.2404x over previous
"""BiLevelRoutingAttention (spiking) Trainium2 kernel, v2.

Sharding: one (t, b) pair per core (T=4 x B=2 = 8 cores). Routing region mean
is an AllReduce among the 4 cores sharing each b.

v2 vs v1:
  - fp8e4 DoubleRow matmuls for qkv / projection (0.5 cyc/col; the 2-way
    channel-half contraction folds into one instruction). Grams also fp8 DR.
  - spike thresholds normalized into the weights (x@w+b >= 2  <=>
    x@(128*w/(2-b)) >= 128), so thresholding is a constant-scalar compare
    split across DVE (is_ge) and ScalarE (saturated sigmoid), one batched
    [128,1024] drain per PSUM pair.
  - gram combine stays on-chip: SBUF->SBUF DMA transposes to window-major and
    back; block-diag bmask folded into the gram PSUM drain.
  - attention divide: rank-1 eps matmul folds +1e-6 into den, DVE
    reciprocal_approx_fast from PSUM, ScalarE copy + GpSimd multiply.
  - projection result DMAd straight from PSUM as f32; host applies
    /64 (fp8 weight scale) and + b_proj.
"""

import numpy as np
import ml_dtypes

T, B, Lt, Lh, Lw, C = 4, 2, 8, 32, 32, 256
WT, WH, WW = 2, 4, 4
LT, LH, LW = Lt // WT, Lh // WH, Lw // WW  # 4, 8, 8
W = WT * WH * WW        # 32 windows
S = LT * LH * LW        # 256 tokens per window
NTOK = W * S            # 8192
H, D = 8, 32
TOPK = 4
NCORES = 8
GROUPS = [[0, 1, 2, 3], [4, 5, 6, 7]]
BF16 = ml_dtypes.bfloat16
WSCALE = 128.0          # spike thresholds scaled to this constant
PSCALE = 64.0           # w_proj scaled by this (undone on host)
SIGK = 1e6              # sigmoid sharpness for ScalarE spike compare
CCH = 344               # combine chunk cols (8256 = 24*344)

_CACHE = {}


def build_kernel():
    from concourse import bacc
    import concourse.mybir as mybir
    import concourse.tile as tile
    from concourse.tile_rust import add_dep_helper
    from concourse.masks import make_identity

    bf = mybir.dt.bfloat16
    f32 = mybir.dt.float32
    f8 = mybir.dt.float8e4
    DR = mybir.MatmulPerfMode.DoubleRow

    nc = bacc.Bacc("TRN2", target_bir_lowering=False, debug=False,
                   num_devices=NCORES)

    xT = nc.dram_tensor("xT", [2, 128, NTOK], bf, kind="ExternalInput")
    x8d = nc.dram_tensor("x8", [2, 128, NTOK], f8, kind="ExternalInput")
    wq = nc.dram_tensor("wq", [128, 2, 2, 128], f8, kind="ExternalInput")
    wkv = nc.dram_tensor("wkv", [128, 2, 512], f8, kind="ExternalInput")
    wproj = nc.dram_tensor("wproj", [128, 2, 2, 128], f8, kind="ExternalInput")
    bmask = nc.dram_tensor("bmask", [128, 2, 129], bf, kind="ExternalInput")
    outT = nc.dram_tensor("outT", [2, 128, NTOK], bf, kind="ExternalOutput")
    sel_dbg = nc.dram_tensor("sel_dbg", [32, 32], f32, kind="ExternalOutput")

    cc_in = nc.dram_tensor("cc_in", [128, 64], f32)
    cc_out = nc.dram_tensor("cc_out", [128, 64], f32)

    with tile.TileContext(nc) as tc:
        with (
            tc.tile_pool(name="big", bufs=1) as big_pool,
            tc.tile_pool(name="persist", bufs=1) as pp,
            tc.tile_pool(name="kvs", bufs=4) as kv_pool,
            tc.tile_pool(name="small", bufs=2) as sm_pool,
            tc.tile_pool(name="outp", bufs=3) as out_pool,
            tc.tile_pool(name="mm512", bufs=2, space="PSUM") as mm512,
            tc.tile_pool(name="big2", bufs=3, space="PSUM") as bp2,
            tc.tile_pool(name="dram", bufs=1, space="DRAM") as dram_pool,
        ):
            gram_dram = dram_pool.tile([32, 128, 258], bf)
            kvr_dram = dram_pool.tile([32, 128, 258], bf)
            # ---- big persistent SBUF tiles ----
            xsb = big_pool.tile([128, 2, NTOK], bf, tag="xsb")
            x8 = big_pool.tile([128, 2, NTOK], f8, tag="x8")
            qsb = big_pool.tile([128, 2, NTOK], bf, tag="qsb")
            attn_nb = big_pool.tile([128, 2, NTOK], f8, tag="attnb")
            gram_all = big_pool.tile([128, 32, 258], bf, tag="gram")
            wmaj = big_pool.tile([128, 8256], bf, tag="wmaj")
            kvr_wmaj = big_pool.tile([128, 8256], bf, tag="kvrw")
            kvread = big_pool.tile([128, 32, 258], bf, tag="kvread")
            dexp = big_pool.tile([128, 2, 32, 128], bf, tag="dexp")

            # ---- input DMAs (sliced so compute can start early) ----
            for p in range(4):
                sl = slice(p * 2048, (p + 1) * 2048)
                for c in range(2):
                    nc.sync.dma_start(x8[:, c, sl], x8d[c, :, sl])
                for c in range(2):
                    nc.sync.dma_start(xsb[:, c, sl], xT[c, :, sl])

            # ---- weights / constants ----
            wq_sb = pp.tile([128, 2, 2, 128], f8)
            nc.sync.dma_start(wq_sb[:], wq[:])
            wkv_sb = pp.tile([128, 2, 512], f8)
            nc.sync.dma_start(wkv_sb[:], wkv[:])
            wproj_sb = pp.tile([128, 2, 2, 128], f8)
            nc.sync.dma_start(wproj_sb[:], wproj[:])
            bmask_sb = pp.tile([128, 2, 129], bf)
            nc.sync.dma_start(bmask_sb[:], bmask[:])
            id32 = pp.tile([32, 32], f32)
            make_identity(nc, id32[:])
            sigb = pp.tile([128, 1], f32)
            nc.vector.memset(sigb[:], -WSCALE * SIGK)
            epsrow = pp.tile([1, 128], bf)
            nc.vector.memset(epsrow[:], 1e-6)
            ones256 = pp.tile([1, 256], bf)
            nc.vector.memset(ones256[:], 1.0)
            ones_kvt = pp.tile([128, 2, 1], f8)
            nc.vector.memset(ones_kvt[:], 1.0)

            # ---- region partial sums (per x slice) -> collective ----
            region = sm_pool.tile([128, 2, 32], f32, tag="region", bufs=1)
            for p in range(4):
                sl = slice(p * 2048, (p + 1) * 2048)
                for c in range(2):
                    nc.vector.reduce_sum(
                        region[:, c, p * 8:(p + 1) * 8],
                        xsb[:, c, sl].rearrange("p (w s) -> p w s", s=S),
                        axis=mybir.AxisListType.X,
                    )
            st = nc.sync.dma_start(cc_in[:], region[:].rearrange("p a w -> p (a w)"))
            cc = nc.gpsimd.collective_compute(
                "AllReduce", mybir.AluOpType.add, replica_groups=GROUPS,
                ins=[cc_in[:]], outs=[cc_out[:]],
            )
            add_dep_helper(cc.ins, st.ins, reason="region stored before collective")
            xs_sb = sm_pool.tile([128, 2, 32], f32, tag="xsum", bufs=1)
            ld = nc.sync.dma_start(xs_sb[:], cc_out[:].rearrange("p (a w) -> p a w", w=32))
            add_dep_helper(ld.ins, cc.ins, reason="collective before readback")

            bmflat = bmask_sb[:].rearrange("p c e -> p (c e)")

            # ---- qkv + spikes + per-window grams (16 blocks of 512 toks) ----
            for blk in range(16):
                tsl = slice(blk * 512, (blk + 1) * 512)
                # q: both halves into one 2-bank PSUM tile, one DVE drain
                qp = bp2.tile([128, 1024], f32, tag="big2")
                for qc in range(2):
                    nc.tensor.matmul(qp[:, 512 * qc:512 * (qc + 1)],
                                     wq_sb[:, :, qc, :], x8[:, :, tsl],
                                     start=True, stop=True, perf_mode=DR)
                nc.vector.tensor_scalar(
                    qsb[:, :, tsl],
                    qp[:].rearrange("p (a t) -> p a t", a=2),
                    WSCALE, None, op0=mybir.AluOpType.is_ge)
                # kv: per window, 2 token-chunk matmuls into one tile,
                # one ScalarE sigmoid drain into the fp8 kvt tile
                kvts = {}
                for wi in range(2):
                    kvt = kv_pool.tile([128, 2, 512], f8, tag="kvt")
                    kvts[wi] = kvt
                    kvp = bp2.tile([128, 1024], f32, tag="big2")
                    for ci in range(2):
                        tci = blk * 4 + wi * 2 + ci
                        ksl = slice(tci * 128, (tci + 1) * 128)
                        nc.tensor.matmul(kvp[:, 512 * ci:512 * (ci + 1)],
                                         x8[:, :, ksl], wkv_sb[:],
                                         start=True, stop=True, perf_mode=DR)
                    nc.scalar.activation(
                        kvt[:, :, 0:512],
                        kvp[:].rearrange("p (a t) -> p a t", a=2),
                        mybir.ActivationFunctionType.Sigmoid,
                        bias=sigb[:], scale=SIGK)
                # grams for the two windows into one tile, one masked drain
                gp = bp2.tile([128, 1024], f32, tag="big2")
                for wi in range(2):
                    kvt = kvts[wi]
                    for c in range(2):
                        off = 512 * wi + 129 * c
                        nc.tensor.matmul(gp[:, off:off + 128],
                                         kvt[:, :, 128 * c:128 * (c + 1)],
                                         kvt[:, :, 256 + 128 * c:384 + 128 * c],
                                         start=True, stop=True, perf_mode=DR)
                        nc.tensor.matmul(gp[:, off + 128:off + 129],
                                         kvt[:, :, 128 * c:128 * (c + 1)],
                                         ones_kvt[:],
                                         start=True, stop=True, perf_mode=DR)
                nc.vector.tensor_tensor(
                    gram_all[:, 2 * blk:2 * blk + 2, :],
                    gp[:].rearrange("p (w x) -> p w x", w=2)[:, :, 0:258],
                    bmflat[:, None, :].to_broadcast([128, 2, 258]),
                    op=mybir.AluOpType.mult)
                nc.sync.dma_start(
                    gram_dram[2 * blk:2 * blk + 2, :, :].rearrange("w p e -> p w e"),
                    gram_all[:, 2 * blk:2 * blk + 2, :])

            # ---- scores -> top-4 selection matrix sel^T (replicated x4) ----
            scpt = mm512.tile([128, 512], f32, tag="mm512")
            scp = scpt[0:32, :]
            for c in range(2):
                nc.tensor.matmul(scp[:, 0:32], xs_sb[:, c, :], xs_sb[:, c, :],
                                 start=(c == 0), stop=(c == 1))
            shifted = sm_pool.tile([32, 32], f32, tag="shifted")
            nc.vector.tensor_scalar(shifted[:], scp[:, 0:32], 1e6, None,
                                    op0=mybir.AluOpType.add)
            mx8 = sm_pool.tile([32, 8], f32, tag="mx8")
            nc.vector.max(mx8[:], shifted[:])
            nc.vector.memset(mx8[:, TOPK:], 0.0)
            zapped = sm_pool.tile([32, 32], f32, tag="zapped")
            nc.vector.match_replace(out=zapped[:], in_to_replace=mx8[:],
                                    in_values=shifted[:], imm_value=0.0)
            selb = sm_pool.tile([32, 32], f32, tag="selb")
            nc.vector.tensor_tensor(selb[:], shifted[:], zapped[:],
                                    op=mybir.AluOpType.is_gt)
            nc.sync.dma_start(sel_dbg[:], selb[:])
            selT_pt = mm512.tile([128, 512], f32, tag="mm512")
            nc.tensor.transpose(selT_pt[0:32, 0:32], selb[:], id32[:])
            selT = sm_pool.tile([32, 32], bf, tag="selT")
            nc.vector.tensor_copy(selT[:], selT_pt[0:32, 0:32])
            selT4 = pp.tile([128, 32], bf)
            for j in range(4):
                nc.sync.dma_start(selT4[32 * j:32 * (j + 1), :], selT[:])

            # ---- window-major gram read (via DRAM round trip) ----
            for a in range(4):
                psl = slice(32 * a, 32 * (a + 1))
                nc.sync.dma_start(
                    wmaj[psl, :].rearrange("j (pl e) -> j pl e", e=258),
                    gram_dram[:, psl, :])

            # ---- combine: kvr rows = selT.T @ gram rows (4-block packed) ----
            for grp in range(24):
                csl = slice(grp * CCH, (grp + 1) * CCH)
                cpt = mm512.tile([128, 512], f32, tag="mm512")
                cp = cpt[:, 0:CCH]
                for a in range(4):
                    psl = slice(32 * a, 32 * (a + 1))
                    nc.tensor.matmul(cp[psl, :], selT4[psl, :], wmaj[psl, csl],
                                     start=True, stop=True,
                                     tile_position=(32 * a, 32 * a))
                if grp % 2 == 0:
                    nc.vector.tensor_copy(kvr_wmaj[:, csl], cp[:])
                else:
                    nc.scalar.activation(kvr_wmaj[:, csl], cp[:],
                                         mybir.ActivationFunctionType.Copy)

            # ---- transpose back via DRAM: kvread[kd, w, (c e)] ----
            for a in range(4):
                psl = slice(32 * a, 32 * (a + 1))
                nc.sync.dma_start(
                    kvr_dram[:, psl, :],
                    kvr_wmaj[psl, :].rearrange("w (pl e) -> w pl e", e=258))
            for a in range(4):
                psl = slice(32 * a, 32 * (a + 1))
                nc.sync.dma_start(
                    kvread[psl, :, :],
                    kvr_dram[:, psl, :].rearrange("w p e -> p w e"))

            # ---- dexp: ksum column broadcast, block-diag masked ----
            for c in range(2):
                nc.vector.tensor_tensor(
                    dexp[:, c, :, :],
                    kvread[:, :, 129 * c + 128:129 * c + 129].to_broadcast([128, 32, 128]),
                    bmask_sb[:, 0:1, 0:128].to_broadcast([128, 32, 128]),
                    op=mybir.AluOpType.mult,
                )

            # ---- attention (out, den) + divide; projection; out DMA ----
            for blk in range(16):
                tsl = slice(blk * 512, (blk + 1) * 512)
                adp_o = bp2.tile([128, 1024], f32, tag="big2")
                adp_d = bp2.tile([128, 1024], f32, tag="big2")
                for wi in range(2):
                    w = blk * 2 + wi
                    wsl = slice(w * 256, (w + 1) * 256)
                    for c in range(2):
                        qtr = slice(512 * wi + 256 * c, 512 * wi + 256 * (c + 1))
                        nc.tensor.matmul(adp_o[:, qtr],
                                         kvread[:, w, 129 * c:129 * c + 128],
                                         qsb[:, c, wsl], start=True, stop=True)
                        nc.tensor.matmul(adp_d[:, qtr], dexp[:, c, w, :],
                                         qsb[:, c, wsl], start=True, stop=False)
                        nc.tensor.matmul(adp_d[:, qtr], epsrow[:],
                                         ones256[:], start=False, stop=True)
                rsb = out_pool.tile([128, 1024], f32, tag="rsb")
                nc.vector.reciprocal_approx_fast(out=rsb[:], in_=adp_d[:])
                attc = out_pool.tile([128, 1024], f8, tag="attc")
                nc.scalar.activation(attc[:], adp_o[:],
                                     mybir.ActivationFunctionType.Copy)
                nc.gpsimd.tensor_tensor(
                    attn_nb[:, :, tsl].rearrange("p c (w s) -> p w c s", w=2),
                    attc[:].rearrange("p (w c s) -> p w c s", w=2, c=2),
                    rsb[:].rearrange("p (w c s) -> p w c s", w=2, c=2),
                    op=mybir.AluOpType.mult)
                for pc in range(2):
                    pjp = mm512.tile([128, 512], f32, tag="mm512")
                    nc.tensor.matmul(pjp[:], wproj_sb[:, :, pc, :],
                                     attn_nb[:, :, tsl],
                                     start=True, stop=True, perf_mode=DR)
                    osb = out_pool.tile([128, 512], bf, tag="osb")
                    if pc == 0:
                        nc.scalar.activation(osb[:], pjp[:],
                                             mybir.ActivationFunctionType.Copy)
                    else:
                        nc.vector.tensor_copy(osb[:], pjp[:])
                    nc.sync.dma_start(outT[pc, :, tsl], osb[:])

    nc.compile()
    return nc


def prep_inputs(x, w_qkv, b_qkv, w_proj, b_proj):
    from concourse import mybir
    f8np = mybir.dt.np(mybir.dt.float8e4)
    th = 2.0 - b_qkv                       # all positive (~2)
    wn = w_qkv * (WSCALE / th)             # spike threshold becomes WSCALE
    wq_a = wn[:, 0:256].reshape(2, 128, 2, 128).transpose(1, 0, 2, 3)
    wkv_a = wn[:, 256:768].reshape(2, 128, 512).transpose(1, 0, 2)
    wproj_a = (w_proj * PSCALE).reshape(2, 128, 2, 128).transpose(1, 0, 2, 3)
    i = np.arange(128)[:, None]
    j = np.arange(129)[None, :]
    bm = ((i // 32) == (j // 32)) | (j == 128)
    bmask_a = np.broadcast_to(bm[:, None, :], (128, 2, 129))
    return {
        "wq": np.ascontiguousarray(wq_a).astype(f8np),
        "wkv": np.ascontiguousarray(wkv_a).astype(f8np),
        "wproj": np.ascontiguousarray(wproj_a).astype(f8np),
        "bmask": np.ascontiguousarray(bmask_a).astype(BF16),
    }, f8np


def window_partition(x):
    """[T,B,Lt,Lh,Lw,C] -> [T,B,NTOK,C] with tokens in (w, s) order."""
    Tb, Bb = x.shape[0], x.shape[1]
    xw = x.reshape(Tb, Bb, WT, LT, WH, LH, WW, LW, C)
    xw = xw.transpose(0, 1, 2, 4, 6, 3, 5, 7, 8)
    return np.ascontiguousarray(xw).reshape(Tb, Bb, NTOK, C)


def window_reverse(o):
    """[NTOK, C] -> [Lt, Lh, Lw, C]."""
    o = o.reshape(WT, WH, WW, LT, LH, LW, C)
    o = o.transpose(0, 3, 1, 4, 2, 5, 6)
    return np.ascontiguousarray(o).reshape(Lt, Lh, Lw, C)


def make_in_maps(inputs):
    x = np.asarray(inputs["x"], dtype=np.float32)
    shared, f8np = prep_inputs(
        x, np.asarray(inputs["w_qkv"], dtype=np.float32),
        np.asarray(inputs["b_qkv"], dtype=np.float32),
        np.asarray(inputs["w_proj"], dtype=np.float32),
        np.asarray(inputs["b_proj"], dtype=np.float32))
    xw = window_partition(x)
    in_maps = []
    for core in range(NCORES):
        b, t = core // 4, core % 4
        xt = np.ascontiguousarray(xw[t, b].T)              # [C, NTOK]
        in_maps.append({
            **shared,
            "xT": xt.reshape(2, 128, NTOK).astype(BF16),
            "x8": xt.reshape(2, 128, NTOK).astype(f8np),
        })
    return in_maps


def run_kernel_spmd(nc, in_maps, **kwargs):
    from concourse.bass_utils import run_bass_kernel_spmd
    return run_bass_kernel_spmd(nc, in_maps, core_ids=list(range(NCORES)), **kwargs)


def collect_out(res, b_proj):
    out = np.empty((T, B, Lt, Lh, Lw, C), dtype=np.float32)
    bp = np.asarray(b_proj, dtype=np.float32)
    for core in range(NCORES):
        b, t = core // 4, core % 4
        oT = res.results[core]["outT"].astype(np.float32).reshape(256, NTOK)
        o = np.ascontiguousarray(oT.T) * (1.0 / PSCALE) + bp
        out[t, b] = window_reverse(o)
    return out


def kernel(x, w_qkv, b_qkv, w_proj, b_proj):
    if "nc" not in _CACHE:
        _CACHE["nc"] = build_kernel()
    nc = _CACHE["nc"]
    in_maps = make_in_maps(dict(x=x, w_qkv=w_qkv, b_qkv=b_qkv,
                                w_proj=w_proj, b_proj=b_proj))
    res = run_kernel_spmd(nc, in_maps)
    return collect_out(res, b_proj)
